# revision 1
# baseline (speedup 1.0000x reference)
"""Trainium2 Bass kernel for nn_Attention (B=4, N=2048, C=768, H=8).

reference:
    qkv = x.reshape(B,N,H,d).transpose(0,2,1,3)      # q=k=v
    attn = softmax(q @ k^T / sqrt(d))
    out  = (attn @ v).transpose -> (B,N,C)
    y    = out @ proj_w.T + proj_b
Sharding: 8 cores = 4 batches x 2 query-halves; no collectives.

Since q=k, S = Q Q^T is symmetric, so within each core's own 1024x1024
(query x own-key) block, exp(S^T) tiles below the diagonal are XBAR
DMA-transposed mirrors of tiles above it -- no mm1 and no ScalarE exp
for them.  Keys/values are pre-rotated per core (shard_inputs) so the
core's own block is always local ktiles 0..7 (the SPMD program is
identical on all cores; softmax/PV are permutation-invariant in k).

Per head, q-chunk 1 (q 512..1023) runs FIRST and computes its 16 score
tiles in full: UR (kt 0..3), far (kt 8..15), D2 (kt 4..7).  The four
exp'd UR tiles are mirrored by 4 batched DMA transposes (out[128,4,128]
<- in[128,512], 14ns/XBAR-tile on the DMA engines) into esL, which
q-chunk 0 consumes directly as its kt 4..7 PV operands.  q-chunk 0 only
computes 12 tiles (far + D1).  Per head: 28 exp tiles instead of 32
(ACT -12.5%), 28 mm1 tiles instead of 32 (PE mm1 -12.5%), mirrors cost
only DMA.

Everything is computed transposed ([feature, token]):
  S^T[k,q]   = (K_h^T)[d,ktile].T-contract @ (Q_h^T)[d,q]
  expS       = exp(scale*S^T)                 (ScalarE, PSUM->SBUF bf16)
  O^T[c,q]   = sum_kt [V_h | 1][ktile,97].T @ expS[ktile,q]  (row 96 =
               softmax denom)
  O_norm     = O^T * (1/denom)  (DVE recip off PSUM + copy, GpSimd
               partition_broadcast, DVE mul, HWDGE DMA-repack into six
               128-row c-chunks)
  Y^T[j,q]   = sum_c (W^T)[c_chunk, jtile].T @ O_norm[c_chunk, q] + b

Schedule: software-pipelined: per group, PE runs mm1(g) | mm2(g-1) |
projection fillers; ACT exp(g) follows mm1(g).  The projection of each
q-chunk drains as filler work through later chunks.  PE warmup matmuls
and an exp-table preload run under the initial DMA wait.
PSUM: 2x3 banks score double-buffer + 1 PV accumulator + 1 projection
accumulator = 8 banks.
"""

import sys
import os

for _p in ("/opt/trn_rl_repo",):
    if os.path.isdir(_p) and _p not in sys.path:
        sys.path.insert(0, _p)

import numpy as np
import ml_dtypes

import concourse.bacc as bacc
import concourse.mybir as mybir
import concourse.tile as tile
from concourse.bass import MemorySpace
from concourse.bass_utils import run_bass_kernel_spmd

BF16 = ml_dtypes.bfloat16

B, N, C = 4, 2048, 768
H = 8
D = C // H            # 96
NCORES = 8
QPC = N // 2          # queries per core = 1024
QC = 512              # q chunk (PSUM free size)
NQC = QPC // QC       # 2
KT = N // 128         # 16 key tiles
JT = C // 128         # 6 output-feature tiles
SCALE = float(D) ** -0.5

# per-chunk score-group ktile lists (local ktile ids; 0..7 = own block)
G_QC1 = [[0, 1, 2], [3, 8, 9], [10, 11, 12], [13, 14, 15], [4, 5], [6, 7]]
G_QC0 = [[8, 9, 10], [11, 12, 13], [14, 15, 0], [1, 2, 3]]
FILLERS = {(1, 0): 1, (1, 1): 1, (1, 2): 1, (1, 3): 1, (1, 4): 1, (1, 5): 1,
           (0, 0): 1, (0, 1): 1, (0, 2): 1, (0, 3): 2}
# flush lag is 2 groups: mm1(g) is emitted before mm2(g-2) so the exp
# stream never waits behind PV work

_cache = {}


def build_bass(iters: int = 1, emit_log: list | None = None):
    """Build the SPMD single-core program (same graph on all 8 cores)."""
    nc = bacc.Bacc("TRN2", target_bir_lowering=False, debug=False,
                   num_devices=NCORES)
    if emit_log is not None:
        _mm0 = nc.tensor.matmul
        _act0 = nc.scalar.activation

        def _mm(*a, **kw):
            emit_log.append(("mm", _cur_label[0]))
            return _mm0(*a, **kw)

        def _act(*a, **kw):
            emit_log.append(("act", _cur_label[0]))
            return _act0(*a, **kw)
        nc.tensor.matmul = _mm
        nc.scalar.activation = _act
    _cur_label = [""]

    def set_label(s):
        _cur_label[0] = s
    f32 = mybir.dt.float32
    bf16 = mybir.dt.bfloat16

    qt = nc.declare_dram_parameter("qt", [D, H, QPC], bf16, isOutput=False)
    kt = nc.declare_dram_parameter("kt", [D, H, N], bf16, isOutput=False)
    vn = nc.declare_dram_parameter("vn", [128, KT, H, D + 1], bf16,
                                   isOutput=False)
    wt = nc.declare_dram_parameter("wt", [128, JT, C], bf16, isOutput=False)
    bias = nc.declare_dram_parameter("bias", [128, JT], f32, isOutput=False)
    out = nc.declare_dram_parameter("out", [C, QPC], f32, isOutput=True)

    with tile.TileContext(nc) as tc:
        with (
            tc.tile_pool(name="consts", bufs=1) as consts,
            tc.tile_pool(name="expp", bufs=8) as expp,
            tc.tile_pool(name="esl", bufs=3) as eslp,
            tc.tile_pool(name="small", bufs=8) as small,
            tc.tile_pool(name="onorm", bufs=2 * JT + 4) as onormp,
            tc.tile_pool(name="ysb", bufs=6) as ysbp,
            tc.tile_pool(name="ps_s", bufs=2, space=MemorySpace.PSUM) as ps_s,
            tc.tile_pool(name="ps_o", bufs=1, space=MemorySpace.PSUM) as ps_o,
            tc.tile_pool(name="ps_y", bufs=1, space=MemorySpace.PSUM) as ps_y,
        ):
            # ---- load constants (consolidated tiles; first-needed
            # first, few large issues so the in-order SP queue reaches
            # head 0's mirror transposes quickly) ----
            qtall = consts.tile([D, H, QPC], bf16, tag="qt", name="qtall")
            ktall = consts.tile([D, H, N], bf16, tag="kt", name="ktall")
            vnall = consts.tile([128, KT, H, D + 1], bf16, tag="vn",
                                name="vnall")
            wtall = consts.tile([128, JT, C], bf16, tag="wt", name="wtall")
            biasall = consts.tile([128, JT], f32, tag="bias", name="biasall")
            nc.sync.dma_start(out=qtall[:, 0, QC:], in_=qt[:, 0, QC:])
            nc.sync.dma_start(out=ktall[:, 0, 0:512], in_=kt[:, 0, 0:512])
            nc.sync.dma_start(out=vnall[:, 0:4], in_=vn[:, 0:4])
            nc.sync.dma_start(out=ktall[:, 0, 1024:], in_=kt[:, 0, 1024:])
            nc.sync.dma_start(out=vnall[:, 8:12], in_=vn[:, 8:12])
            nc.sync.dma_start(out=qtall[:, 0, 0:QC], in_=qt[:, 0, 0:QC])
            nc.sync.dma_start(out=ktall[:, 0, 512:1024],
                              in_=kt[:, 0, 512:1024])
            nc.sync.dma_start(out=vnall[:, 12:16], in_=vn[:, 12:16])
            nc.sync.dma_start(out=vnall[:, 4:8], in_=vn[:, 4:8])
            nc.sync.dma_start(out=qtall[:, 1], in_=qt[:, 1])
            nc.sync.dma_start(out=ktall[:, 1], in_=kt[:, 1])
            nc.sync.dma_start(out=qtall[:, 2:8], in_=qt[:, 2:8])
            nc.sync.dma_start(out=ktall[:, 2:5], in_=kt[:, 2:5])
            nc.sync.dma_start(out=ktall[:, 5:8], in_=kt[:, 5:8])
            nc.sync.dma_start(out=wtall[:], in_=wt[:])
            nc.sync.dma_start(out=biasall[:], in_=bias[:])

            # HAM warmup: dummy matmuls with no input deps keep the PE
            # activity monitor busy during the initial DMA wait.
            wz = consts.tile([D, QC], bf16, tag="wz", name="wz")
            nc.vector.memset(wz[:], 0)
            pyw = ps_y.tile([128, QC], f32, tag="py", name="pyw")
            set_label("warmup")
            for _w in range(8):
                nc.tensor.matmul(pyw[:], wz[:, 0:128], wz[:],
                                 start=True, stop=True)
            # preload the ScalarE exp table set during the DMA wait
            wze = small.tile([1, 16], bf16, tag="wze", name="wze")
            nc.scalar.activation(out=wze[:], in_=wz[0:1, 0:16],
                                 func=mybir.ActivationFunctionType.Exp)

            from collections import deque

            pend = deque()       # (po, h, cnt, items, fin), up to 2 deep
            projq = deque()

            def flush_one():
                if not pend:
                    return
                po, h, cnt, items, fin = pend.popleft()
                set_label(f"mm2 h{h}")
                for es_ap, t_i in items:
                    nc.tensor.matmul(
                        po[:], vnall[:, t_i, h, :], es_ap,
                        start=(cnt[0] == 0), stop=(cnt[0] == KT - 1),
                    )
                    cnt[0] += 1
                if fin is not None:
                    fin()

            def flush_pend():
                while pend:
                    flush_one()

            def emit_one_proj():
                if projq:
                    projq.popleft()()

            def emit_chunk(h, qc, oglob, last=False):
                po = ps_o.tile([D + 1, QC], f32, tag="po")
                cnt = [0]
                qoff = qc * QC
                groups = G_QC1 if qc == 1 else G_QC0
                src = {}          # local kt (0..3) -> exp'd es AP (qc1 only)
                esl = [None]

                def normalize():
                    rc = small.tile([1, QC], f32, tag="rc")
                    nc.vector.reciprocal(out=rc[:], in_=po[D:D + 1, :])
                    if last:
                        oc = po
                    else:
                        oc = small.tile([D, QC], f32, tag="oc")
                        nc.vector.tensor_copy(out=oc[:], in_=po[0:D, :])
                    bc = small.tile([D, QC], f32, tag="bc")
                    nc.gpsimd.partition_broadcast(bc[:], rc[:])
                    on = small.tile([D, QC], bf16, tag="on")
                    nc.vector.tensor_mul(on[:], oc[0:D, :], bc[:])
                    a0 = (D * h) % 128
                    c0 = (D * h) // 128
                    s1 = min(128 - a0, D)
                    nc.sync.dma_start(out=oglob[c0][a0:a0 + s1, :],
                                      in_=on[0:s1, :])
                    if s1 < D:
                        nc.sync.dma_start(out=oglob[c0 + 1][0:D - s1, :],
                                          in_=on[s1:D, :])

                for gi, grp in enumerate(groups):
                    ps = ps_s.tile([128, 3, QC], f32, tag="ps")
                    set_label(f"mm1 h{h} qc{qc} g{gi}")
                    for i, t_i in enumerate(grp):
                        nc.tensor.matmul(
                            ps[:, i, :],
                            ktall[:, h, t_i * 128:(t_i + 1) * 128],
                            qtall[:, h, qoff:qoff + QC],
                            start=True, stop=True,
                        )
                    if len(pend) >= 2:
                        flush_one()
                    # mirror transposes for the next chunk, placed after
                    # their source exps
                    if qc == 1 and gi == 1:
                        esl[0] = eslp.tile([128, 4, QC], bf16, tag="esL",
                                           name=f"esL{h}")
                        for b in range(3):
                            nc.sync.dma_start_transpose(
                                out=esl[0][:, :, b * 128:(b + 1) * 128],
                                in_=src[b])
                    elif qc == 1 and gi == 2:
                        nc.sync.dma_start_transpose(
                            out=esl[0][:, :, 384:512], in_=src[3])
                    for _f in range(FILLERS[(qc, gi)]):
                        emit_one_proj()
                    es = expp.tile([128, 3, QC], bf16, tag="es")
                    set_label(f"exp h{h} qc{qc} g{gi}")
                    nc.scalar.activation(
                        out=es[:, 0:len(grp), :], in_=ps[:, 0:len(grp), :],
                        func=mybir.ActivationFunctionType.Exp,
                        scale=SCALE,
                    )
                    items = [(es[:, i, :], t_i) for i, t_i in enumerate(grp)]
                    if qc == 1:
                        for i, t_i in enumerate(grp):
                            if t_i < 4:
                                src[t_i] = es[:, i, :]
                    if qc == 0 and gi == 0:
                        lprev = esl_by_head[h]
                        items += [(lprev[:, j, :], 4 + j) for j in range(4)]
                    fin = normalize if gi == len(groups) - 1 else None
                    pend.append((po, h, cnt, items, fin))
                if qc == 1:
                    esl_by_head[h] = esl[0]

            def queue_proj(qc, oglob):
                py_box = [None]
                for j in range(JT):
                    def mk_mm(j, c):
                        def go():
                            if c == 0:
                                py_box[0] = ps_y.tile([128, QC], f32,
                                                      tag="py", name="py")
                            set_label(f"proj j{j} c{c} qc{qc}")
                            nc.tensor.matmul(
                                py_box[0][:],
                                wtall[:, c, j * 128:(j + 1) * 128],
                                oglob[c][:],
                                start=(c == 0), stop=(c == JT - 1),
                            )
                        return go
                    for c in range(JT):
                        projq.append(mk_mm(j, c))

                    def mk_fin(j):
                        def go():
                            y = ysbp.tile([128, QC], f32, tag="y", name="y")
                            nc.vector.tensor_scalar_add(
                                out=y[:], in0=py_box[0][:],
                                scalar1=biasall[:, j:j + 1],
                            )
                            nc.sync.dma_start(
                                out=out[j * 128:(j + 1) * 128,
                                        qc * QC:(qc + 1) * QC],
                                in_=y[:],
                            )
                        return go
                    projq.append(mk_fin(j))

            esl_by_head = {}
            final_oglob = None
            for it in range(iters):
                og = {q: [onormp.tile([128, QC], bf16, tag="og",
                                      name=f"og{q}_{c}")
                          for c in range(JT)] for q in (1, 0)}
                for h in range(H):
                    for qc in (1, 0):
                        islast = (it == iters - 1 and h == H - 1 and qc == 0)
                        emit_chunk(h, qc, og[qc], last=islast)
                        if h == H - 1:
                            if islast:
                                final_oglob = (0, og[0])
                            else:
                                queue_proj(qc, og[qc])
            flush_pend()
            while projq:
                emit_one_proj()
            # Final q-chunk's projection, c-chunk-major: chunks 0..4 are
            # ready before the last head's repack lands, so PE stays busy
            # and only the last 6 matmuls wait on it.  Uses the freed
            # score slots as two 3-bank accumulators.
            fqc, fog = final_oglob
            accA = ps_s.tile([128, 3, QC], f32, tag="ps", name="accA")
            accB = ps_s.tile([128, 3, QC], f32, tag="ps", name="accB")
            for c in range(JT):
                for j in range(JT):
                    acc = accA if j < 3 else accB
                    set_label(f"fproj j{j} c{c}")
                    nc.tensor.matmul(
                        acc[:, j % 3, :],
                        wtall[:, c, j * 128:(j + 1) * 128],
                        fog[c][:],
                        start=(c == 0), stop=(c == JT - 1),
                    )
            for j in range(JT):
                acc = accA if j < 3 else accB
                y = ysbp.tile([128, QC], f32, tag="y", name=f"yf{j}")
                nc.vector.tensor_scalar_add(
                    out=y[:], in0=acc[:, j % 3, :], scalar1=biasall[:, j:j + 1],
                )
                nc.sync.dma_start(
                    out=out[j * 128:(j + 1) * 128,
                            fqc * QC:(fqc + 1) * QC],
                    in_=y[:],
                )
    nc.compile()
    return nc


def shard_inputs(x, proj_w, proj_b):
    x = np.asarray(x, dtype=np.float32)
    proj_w = np.asarray(proj_w, dtype=np.float32)
    proj_b = np.asarray(proj_b, dtype=np.float32)

    wt_full = np.ascontiguousarray(
        proj_w.T.reshape(JT, 128, C).transpose(1, 0, 2)).astype(BF16)
    bias_full = np.ascontiguousarray(proj_b.reshape(JT, 128).T)

    in_maps = []
    for c in range(NCORES):
        b = c // 2
        q0 = (c % 2) * QPC
        xb = x[b]                                   # (N, C)
        xtb = np.ascontiguousarray(xb.T)            # (C, N)
        # rotate keys/values so this core's own q-range is ktiles 0..7
        # (softmax and PV are permutation-invariant along k)
        xbk = np.roll(xb, -q0, axis=0)
        kt_c = np.ascontiguousarray(
            xbk.T.reshape(H, D, N).transpose(1, 0, 2)).astype(BF16)
        qt_c = np.ascontiguousarray(
            xtb[:, q0:q0 + QPC].reshape(H, D, QPC).transpose(1, 0, 2)
        ).astype(BF16)
        vn_f = np.ones((N, H, D + 1), dtype=np.float32)
        vn_f[:, :, :D] = xbk.reshape(N, H, D)
        vn_f = np.ascontiguousarray(
            vn_f.reshape(KT, 128, H, D + 1).transpose(1, 0, 2, 3))
        in_maps.append({
            "qt": qt_c,
            "kt": kt_c,
            "vn": vn_f.astype(BF16),
            "wt": wt_full,
            "bias": bias_full,
        })
    return in_maps


def assemble(results):
    y = np.empty((B, N, C), dtype=np.float32)
    for c in range(NCORES):
        b = c // 2
        q0 = (c % 2) * QPC
        y[b, q0:q0 + QPC, :] = results[c]["out"].T
    return y


def kernel(x, proj_w, proj_b):
    if "nc" not in _cache:
        _cache["nc"] = build_bass()
    nc = _cache["nc"]
    in_maps = shard_inputs(x, proj_w, proj_b)
    res = run_bass_kernel_spmd(nc, in_maps, core_ids=list(range(NCORES)))
    return assemble(res.results)

